# revision 9
# baseline (speedup 1.0000x reference)
"""Trainium2 Bass kernel: Bahdanau (additive) attention with coverage.

Reference computation (per batch element b, data-parallel over B=8 cores):
    enc   = tanh(enc_raw + cov[:,None]*wcov)            [S,H]
    a1    = dec @ Wq + bq                               [T,H]
    a2    = enc @ Wc                                    [S,H]
    scores[t,s] = sum_h v[h] * tanh(a1[t,h] + a2[s,h])  [T,S]
    align = softmax(scores, -1)                         [T,S]
    c     = align @ enc                                 [T,H]
    attn_h = [c, dec] @ Wo + bo                         [T,H]
Outputs: attn_h -> [T,B,H], align -> [T,B,S].

Two device variants:
  exact   : big-buffer DVE outer-sum (a1+a2) + ACT tanh + PE v-dot into a
            single [64,512] PSUM scores tile (sliding-window masked-v lhsT).
  fourier : tanh(z) ~ sum_m b_m sin(m*pi/L * z); angle addition splits the
            score into PE matmuls over 2M sin/cos feature maps, ACT only
            computes the features (DVE does range reduction mod 2pi).
"""

import os

import numpy as np

T, B, S, H = 64, 8, 512, 512
P = 128
KT = H // P  # 4 partition tiles of H

VARIANT = os.environ.get("ATTN_VARIANT", "exact")  # "exact" | "fourier"
PROBES = os.environ.get("ATTN_PROBES", "0") == "1"
TG = 2  # t-group size (exact variant)
M_F = int(os.environ.get("ATTN_M", "16"))  # number of sine harmonics (fourier)
L_F = 11.0  # half-period of the sine basis
TWO_PI = float(2 * np.pi)

FREQ_W = [0.2837544416, 0.8559440962, 1.4398952546, 2.0384750571,
          2.6518142442, 3.2791112395, 3.9194354832, 4.5716932015,
          5.2332940231, 5.8897412933]
FREQ_B = [1.2338354998, 0.3228807754, 0.1244921527, 0.0493901935,
          0.0192673255, 0.0073474932, 0.0027404072, 0.001000557,
          0.0003570621, 0.0001197961]

_BUILT = {}
LAST_RESULT = None


def _sine_coefs():
    z = np.linspace(-9.0, 9.0, 4001)
    A = np.sin(np.pi / L_F * np.outer(z, np.arange(1, M_F + 1)))
    b, *_ = np.linalg.lstsq(A, np.tanh(z), rcond=None)
    return b


def _emit(nc, tc, ctx, din, dout, variant):
    import concourse.mybir as mybir

    f32 = mybir.dt.float32
    AF = mybir.ActivationFunctionType
    ALU = mybir.AluOpType
    AX = mybir.AxisListType

    f32r = mybir.dt.float32r

    def r(ap):
        return ap.bitcast(f32r)

    pers = ctx.enter_context(tc.tile_pool(name="pers", bufs=1))
    big = ctx.enter_context(tc.tile_pool(name="big", bufs=3))
    pb2 = ctx.enter_context(tc.tile_pool(name="pb2", bufs=2))
    pb4 = ctx.enter_context(tc.tile_pool(name="pb4", bufs=4))
    psT = ctx.enter_context(tc.tile_pool(name="psT", bufs=2, space="PSUM"))
    psSm = ctx.enter_context(tc.tile_pool(name="psSm", bufs=2, space="PSUM"))
    psOut = ctx.enter_context(tc.tile_pool(name="psOut", bufs=1, space="PSUM"))

    def ld(dram_ap, shape, tag):
        t = pers.tile(shape, f32, tag=tag)
        nc.sync.dma_start(out=t[:], in_=dram_ap)
        return t

    wq = [ld(din["wq"][k * P:(k + 1) * P, :], [P, H], f"wq{k}") for k in range(KT)]
    wc = [ld(din["wc"][k * P:(k + 1) * P, :], [P, H], f"wc{k}") for k in range(KT)]
    wo = [ld(din["wo"][k * P:(k + 1) * P, :], [P, H], f"wo{k}") for k in range(2 * KT)]
    decT = [ld(din["decT"][k * P:(k + 1) * P, :], [P, T], f"decT{k}") for k in range(KT)]
    enc = [ld(din["enc"][j * P:(j + 1) * P, :], [P, H], f"enc{j}") for j in range(KT)]
    encT_all = pb4.tile([P, KT * S], f32, tag="u2")
    encT = [encT_all[:, i * S:(i + 1) * S] for i in range(KT)]
    for i in range(KT):
        nc.sync.dma_start(out=encT[i], in_=din["encT"][i * P:(i + 1) * P, :])
    vwin = [ld(din["vwin"][k], [P, 127], f"vwin{k}") for k in range(KT)]
    covr = ld(din["cov"][:], [1, S], "covr")
    wcovr = ld(din["wcov"][:], [1, H], "wcovr")
    bqr = ld(din["bq"][:], [1, H], "bqr")
    bor = ld(din["bo"][:], [1, H], "bor")
    eye64 = ld(din["eye64"][:], [T, T], "eye64")
    ones64 = pers.tile([1, T], f32, tag="ones64")
    nc.vector.memset(ones64[:], 1.0)

    if PROBES:
        pr = ld(din["probe"][:], [P, T], "probe")
        ps = pers.tile([P, T], f32, tag="probe_sin")
        nc.scalar.activation(ps[:], pr[:], AF.Sin)
        nc.sync.dma_start(out=dout["probe_sin"][:], in_=ps[:])
        pm = pers.tile([P, T], f32, tag="probe_mod")
        ki = pers.tile([P, T], mybir.dt.int32, tag="probe_ki")
        nc.vector.tensor_scalar(ki[:], pr[:], float(1.0 / (2 * np.pi)), None, ALU.mult)
        nc.vector.tensor_copy(pm[:], ki[:])
        nc.sync.dma_start(out=dout["probe_mod"][:], in_=pm[:])

    # coverage: enc = tanh(enc_raw + cov (x) wcov) in both layouts
    for j in range(KT):  # [S,H] layout: outer[s,h] = cov[s]*wcov[h]
        op = psT.tile([P, H], f32, tag="pt")
        nc.tensor.matmul(op[:], covr[0:1, j * P:(j + 1) * P], wcovr[0:1, :],
                         start=True, stop=True)
        nc.vector.tensor_add(enc[j][:], enc[j][:], op[:])
        nc.scalar.activation(enc[j][:], enc[j][:], AF.Tanh)
    for i in range(KT):  # [H,S] layout
        op = psT.tile([P, S], f32, tag="pt")
        nc.tensor.matmul(op[:], wcovr[0:1, i * P:(i + 1) * P], covr[0:1, :],
                         start=True, stop=True)
        nc.vector.tensor_add(encT[i], encT[i], op[:])
        nc.scalar.activation(encT[i], encT[i], AF.Tanh)

    # a2T[hout, s] = sum_hin Wc[hin,hout] * encT[hin,s]   (one [128, KT*S] tile)
    a2T = pers.tile([P, KT * S], f32, tag="a2T")
    for m in range(KT):
        pm2 = psT.tile([P, S], f32, tag="pt")
        for k in range(KT):
            nc.tensor.matmul(pm2[:], wc[k][:, m * P:(m + 1) * P], encT[k],
                             start=(k == 0), stop=(k == KT - 1))
        nc.vector.tensor_copy(a2T[:, m * S:(m + 1) * S], pm2[:])

    # a1T[hout, t] = sum_hin Wq[hin,hout] * decT[hin,t] + bq[hout]
    a1T = pers.tile([P, KT * T], f32, tag="a1T")
    for m in range(KT):
        pm1 = psSm.tile([P, T], f32, tag="ps")
        for k in range(KT):
            nc.tensor.matmul(pm1[:], wq[k][:, m * P:(m + 1) * P], decT[k][:],
                             start=(k == 0), stop=False)
        nc.tensor.matmul(pm1[:], bqr[0:1, m * P:(m + 1) * P], ones64[0:1, :],
                         start=False, stop=True)
        nc.vector.tensor_copy(a1T[:, m * T:(m + 1) * T], pm1[:])

    scores = psOut.tile([T, S], f32, tag="out512")

    if variant == "exact":
        vwinr = []
        for k in range(KT):
            vr = pers.tile([P, 127], f32r, tag=f"vwinr{k}")
            nc.vector.tensor_copy(vr[:].bitcast(f32r), vwin[k][:])
            vwinr.append(vr)
        NG = T // TG
        first = True
        for g in range(NG):
            bb = big.tile([P, KT * TG * S], f32, tag="bb")
            for k in range(KT):
                for tt in range(TG):
                    t_ = g * TG + tt
                    seg = bb[:, (k * TG + tt) * S:(k * TG + tt + 1) * S]
                    nc.vector.tensor_scalar_add(
                        seg, a2T[:, k * S:(k + 1) * S],
                        a1T[:, k * T + t_:k * T + t_ + 1])
            tb = big.tile([P, KT * TG * S], f32r, tag="tb")
            nc.scalar.activation(tb[:], bb[:], AF.Tanh)
            for k in range(KT):
                for tt in range(TG):
                    t_ = g * TG + tt
                    seg = tb[:, (k * TG + tt) * S:(k * TG + tt + 1) * S]
                    last = (g == NG - 1 and k == KT - 1 and tt == TG - 1)
                    nc.tensor.matmul(scores[:], r(vwinr[k][:, 63 - t_:127 - t_]), seg,
                                     start=first, stop=last)
                    first = False
    else:  # fourier
        MF = len(FREQ_W)
        OFF = float(12 * np.pi)  # multiple of 2*pi; makes mod input positive
        HALF_PI = float(np.pi / 2)
        negpi = pers.tile([P, 1], f32, tag="negpi")
        nc.vector.memset(negpi[:], float(-np.pi))
        # a1-side features, batched over all frequencies: [128, MF*KT*T]
        FW = KT * T
        y1a = pb2.tile([P, MF * FW], f32, tag="tmp1")
        for i in range(MF):
            nc.vector.tensor_scalar(y1a[:, i * FW:(i + 1) * FW], a1T[:],
                                    float(FREQ_W[i]), OFF, ALU.mult, ALU.add)
        u1s = pb2.tile([P, MF * FW], f32, tag="tmp1")
        s1a = pers.tile([P, MF * FW], f32, tag="s1a")
        nc.vector.tensor_scalar(u1s[:], y1a[:], TWO_PI, None, ALU.mod)
        nc.scalar.activation(s1a[:], u1s[:], AF.Sin, bias=negpi[:])
        u1c = pb2.tile([P, MF * FW], f32, tag="tmp1")
        c1a = pers.tile([P, MF * FW], f32, tag="c1a")
        nc.vector.tensor_scalar(u1c[:], y1a[:], HALF_PI, TWO_PI, ALU.add, ALU.mod)
        nc.scalar.activation(c1a[:], u1c[:], AF.Sin, bias=negpi[:])
        for i in range(MF):
            wi = float(FREQ_W[i])
            bi = float(FREQ_B[i])
            # a2-side features (sign-flipped: sin(u - pi) = -sin(arg))
            y2 = pb2.tile([P, KT * S], f32, tag="y2")
            nc.vector.tensor_scalar(y2[:], a2T[:], wi, OFF, ALU.mult, ALU.add)
            u2s = pb4.tile([P, KT * S], f32, tag="u2")
            nc.vector.tensor_scalar(u2s[:], y2[:], TWO_PI, None, ALU.mod)
            s2 = pb4.tile([P, KT * S], f32, tag="f2")
            nc.scalar.activation(s2[:].bitcast(f32r), u2s[:], AF.Sin, bias=negpi[:])
            u2c = pb4.tile([P, KT * S], f32, tag="u2")
            nc.vector.tensor_scalar(u2c[:], y2[:], HALF_PI, TWO_PI, ALU.add, ALU.mod)
            c2 = pb4.tile([P, KT * S], f32, tag="f2")
            nc.scalar.activation(c2[:].bitcast(f32r), u2c[:], AF.Sin, bias=negpi[:])
            # scale a1 features by v[h]*b_i per hout chunk
            ws1 = pb4.tile([P, FW], f32, tag="wf1")
            wc1 = pb4.tile([P, FW], f32, tag="wf1")
            for k in range(KT):
                sl = slice(k * T, (k + 1) * T)
                nc.vector.tensor_scalar(ws1[:, sl].bitcast(f32r),
                                        s1a[:, i * FW + k * T:i * FW + (k + 1) * T],
                                        vwin[k][:, 63:64], bi, ALU.mult, ALU.mult)
                nc.vector.tensor_scalar(wc1[:, sl].bitcast(f32r),
                                        c1a[:, i * FW + k * T:i * FW + (k + 1) * T],
                                        vwin[k][:, 63:64], bi, ALU.mult, ALU.mult)
            for k in range(KT):
                nc.tensor.matmul(scores[:], r(ws1[:, k * T:(k + 1) * T]),
                                 r(c2[:, k * S:(k + 1) * S]),
                                 start=(i == 0 and k == 0), stop=False)
                nc.tensor.matmul(scores[:], r(wc1[:, k * T:(k + 1) * T]),
                                 r(s2[:, k * S:(k + 1) * S]),
                                 start=False, stop=(i == MF - 1 and k == KT - 1))

    # softmax over s (free dim)
    negmax = pers.tile([T, 1], f32, tag="negmax")
    nc.vector.tensor_reduce(negmax[:], scores[:], axis=AX.X, op=ALU.max, negate=True)
    align_sb = pers.tile([T, S], f32, tag="align_sb")
    sums = pers.tile([T, 1], f32, tag="sums")
    nc.scalar.activation(align_sb[:], scores[:], AF.Exp, bias=negmax[:],
                         accum_out=sums[:])
    recips = pers.tile([T, 1], f32, tag="recips")
    nc.vector.reciprocal(recips[:], sums[:])
    nc.vector.tensor_scalar_mul(align_sb[:], align_sb[:], recips[:])
    nc.sync.dma_start(out=dout["align"][:], in_=align_sb[:])

    # alignT via PE transpose
    alignT = []
    for j in range(KT):
        pt = psSm.tile([P, T], f32, tag="ps")
        nc.tensor.transpose(pt[:], align_sb[:, j * P:(j + 1) * P], eye64[:])
        at = pers.tile([P, T], f32, tag=f"alignT{j}")
        nc.vector.tensor_copy(at[:], pt[:])
        alignT.append(at)

    # cT[h, t] = sum_s enc[s,h] * alignT[s,t]
    cT = []
    for m in range(KT):
        pc = psSm.tile([P, T], f32, tag="ps")
        for j in range(KT):
            nc.tensor.matmul(pc[:], enc[j][:, m * P:(m + 1) * P], alignT[j][:],
                             start=(j == 0), stop=(j == KT - 1))
        ct = pers.tile([P, T], f32, tag=f"cT{m}")
        nc.vector.tensor_copy(ct[:], pc[:])
        cT.append(ct)

    # attn_h = [c, dec] @ Wo + bo
    pa = psOut.tile([T, H], f32, tag="out512")
    for k in range(KT):
        nc.tensor.matmul(pa[:], cT[k][:], wo[k][:], start=(k == 0), stop=False)
    for k in range(KT):
        nc.tensor.matmul(pa[:], decT[k][:], wo[KT + k][:], start=False, stop=False)
    nc.tensor.matmul(pa[:], ones64[0:1, :], bor[0:1, :], start=False, stop=True)
    attn_sb = pers.tile([T, H], f32, tag="attn_sb")
    nc.vector.tensor_copy(attn_sb[:], pa[:])
    nc.sync.dma_start(out=dout["attn_h"][:], in_=attn_sb[:])


def build(variant=None):
    variant = variant or VARIANT
    if variant in _BUILT:
        return _BUILT[variant]
    from contextlib import ExitStack

    import concourse.bacc as bacc
    import concourse.mybir as mybir
    import concourse.tile as tile

    f32 = mybir.dt.float32
    nc = bacc.Bacc("TRN2", target_bir_lowering=False, debug=False)
    in_specs = [
        ("decT", [H, T]), ("enc", [S, H]), ("encT", [H, S]),
        ("wq", [H, H]), ("wc", [H, H]), ("wo", [2 * H, H]),
        ("vwin", [KT, P, 127]), ("cov", [1, S]), ("wcov", [1, H]),
        ("bq", [1, H]), ("bo", [1, H]), ("eye64", [T, T]),
    ]
    out_specs = [("attn_h", [T, H]), ("align", [T, S])]
    if PROBES:
        in_specs.append(("probe", [P, T]))
        out_specs += [("probe_sin", [P, T]), ("probe_mod", [P, T])]
    din = {n: nc.declare_dram_parameter(n, s, f32, isOutput=False) for n, s in in_specs}
    dout = {n: nc.declare_dram_parameter(n, s, f32, isOutput=True) for n, s in out_specs}
    with ExitStack() as ctx:
        tc = ctx.enter_context(tile.TileContext(nc))
        _emit(nc, tc, ctx, din, dout, variant)
    nc.compile()
    _BUILT[variant] = nc
    return nc


def prep_core_inputs(inputs):
    """Host-side shard: per-core input dicts (core b <- batch element b)."""
    dec = np.asarray(inputs["attn_dec_state"], np.float32)  # [T,B,H]
    encr = np.asarray(inputs["attn_enc_state"], np.float32)  # [S,B,H]
    cov = np.asarray(inputs["attn_coverage"], np.float32)  # [B,S]
    Wq = np.ascontiguousarray(np.asarray(inputs["Wq"], np.float32))
    Wc = np.ascontiguousarray(np.asarray(inputs["Wc"], np.float32))
    Wo = np.ascontiguousarray(np.asarray(inputs["Wo"], np.float32))
    v = np.asarray(inputs["v"], np.float32)
    bq = np.asarray(inputs["bq"], np.float32)[None, :]
    bo = np.asarray(inputs["bo"], np.float32)[None, :]
    wcov = np.asarray(inputs["wcov"], np.float32)[None, :]
    vwin = np.zeros((KT, P, 127), np.float32)
    for k in range(KT):
        vwin[k, :, 63] = v[k * P:(k + 1) * P]
    eye64 = np.eye(T, dtype=np.float32)
    shared = dict(wq=Wq, wc=Wc, wo=Wo, vwin=vwin, wcov=wcov, bq=bq, bo=bo,
                  eye64=eye64)
    if PROBES:
        shared["probe"] = np.linspace(-16, 16, P * T).astype(np.float32).reshape(P, T)
    maps = []
    for b in range(B):
        e = np.ascontiguousarray(encr[:, b, :])
        maps.append(dict(
            decT=np.ascontiguousarray(dec[:, b, :].T),
            enc=e,
            encT=np.ascontiguousarray(e.T),
            cov=np.ascontiguousarray(cov[b][None, :]),
            **shared,
        ))
    return maps


def kernel(**inputs):
    global LAST_RESULT
    nc = build()
    in_maps = prep_core_inputs(inputs)
    from concourse.bass_utils import run_bass_kernel_spmd

    trace = os.environ.get("ATTN_TRACE", "0") == "1"
    res = run_bass_kernel_spmd(nc, in_maps, list(range(B)), trace=trace)
    LAST_RESULT = res
    attn_h = np.stack([res.results[i]["attn_h"] for i in range(B)], axis=1)
    align = np.stack([res.results[i]["align"] for i in range(B)], axis=1)
    return attn_h, align


# revision 14
# speedup vs baseline: 9419.5926x; 9419.5926x over previous
"""Trainium2 Bass kernel: Bahdanau (additive) attention with coverage.

Reference computation (per batch element b, data-parallel over B=8 cores):
    enc   = tanh(enc_raw + cov[:,None]*wcov)            [S,H]
    a1    = dec @ Wq + bq                               [T,H]
    a2    = enc @ Wc                                    [S,H]
    scores[t,s] = sum_h v[h] * tanh(a1[t,h] + a2[s,h])  [T,S]
    align = softmax(scores, -1)                         [T,S]
    c     = align @ enc                                 [T,H]
    attn_h = [c, dec] @ Wo + bo                         [T,H]
Outputs: attn_h -> [T,B,H], align -> [T,B,S].

Two device variants:
  exact   : big-buffer DVE outer-sum (a1+a2) + ACT tanh + PE v-dot into a
            single [64,512] PSUM scores tile (sliding-window masked-v lhsT).
  fourier : tanh(z) ~ sum_m b_m sin(m*pi/L * z); angle addition splits the
            score into PE matmuls over 2M sin/cos feature maps, ACT only
            computes the features (DVE does range reduction mod 2pi).
"""

import os

import numpy as np

T, B, S, H = 64, 8, 512, 512
P = 128
KT = H // P  # 4 partition tiles of H

VARIANT = os.environ.get("ATTN_VARIANT", "exact")  # "exact" | "fourier"
PROBES = os.environ.get("ATTN_PROBES", "0") == "1"
TG = 2  # t-group size (exact variant)
M_F = int(os.environ.get("ATTN_M", "16"))  # number of sine harmonics (fourier)
L_F = 11.0  # half-period of the sine basis
TWO_PI = float(2 * np.pi)

FREQ_W = [0.2837544416, 0.8559440962, 1.4398952546, 2.0384750571,
          2.6518142442, 3.2791112395, 3.9194354832, 4.5716932015,
          5.2332940231, 5.8897412933]
FREQ_B = [1.2338354998, 0.3228807754, 0.1244921527, 0.0493901935,
          0.0192673255, 0.0073474932, 0.0027404072, 0.001000557,
          0.0003570621, 0.0001197961]

_BUILT = {}
LAST_RESULT = None


def _sine_coefs():
    z = np.linspace(-9.0, 9.0, 4001)
    A = np.sin(np.pi / L_F * np.outer(z, np.arange(1, M_F + 1)))
    b, *_ = np.linalg.lstsq(A, np.tanh(z), rcond=None)
    return b


def _emit(nc, tc, ctx, din, dout, variant):
    import concourse.mybir as mybir

    f32 = mybir.dt.float32
    AF = mybir.ActivationFunctionType
    ALU = mybir.AluOpType
    AX = mybir.AxisListType

    f32r = mybir.dt.float32r

    def r(ap):
        return ap.bitcast(f32r)

    pers = ctx.enter_context(tc.tile_pool(name="pers", bufs=1))
    big = ctx.enter_context(tc.tile_pool(name="big", bufs=3))
    pb2 = ctx.enter_context(tc.tile_pool(name="pb2", bufs=2))
    pb4 = ctx.enter_context(tc.tile_pool(name="pb4", bufs=4))
    psT = ctx.enter_context(tc.tile_pool(name="psT", bufs=2, space="PSUM"))
    psSm = ctx.enter_context(tc.tile_pool(name="psSm", bufs=2, space="PSUM"))
    psOut = ctx.enter_context(tc.tile_pool(name="psOut", bufs=1, space="PSUM"))

    def ld(dram_ap, shape, tag):
        t = pers.tile(shape, f32, tag=tag)
        nc.sync.dma_start(out=t[:], in_=dram_ap)
        return t

    covr = ld(din["cov"][:], [1, S], "covr")
    wcovr = ld(din["wcov"][:], [1, H], "wcovr")
    encT_all = (pb4.tile([P, KT * S], f32, tag="u2") if variant == "fourier"
                else pers.tile([P, KT * S], f32, tag="encT"))
    encT = [encT_all[:, i * S:(i + 1) * S] for i in range(KT)]
    for i in range(KT):
        nc.sync.dma_start(out=encT[i], in_=din["encT"][i * P:(i + 1) * P, :])
    wc = [ld(din["wc"][k * P:(k + 1) * P, :], [P, H], f"wc{k}") for k in range(KT)]
    decT = [ld(din["decT"][k * P:(k + 1) * P, :], [P, T], f"decT{k}") for k in range(KT)]
    wq = [ld(din["wq"][k * P:(k + 1) * P, :], [P, H], f"wq{k}") for k in range(KT)]
    vwin = [ld(din["vwin"][k], [P, 127], f"vwin{k}") for k in range(KT)]
    bqr = ld(din["bq"][:], [1, H], "bqr")
    enc = [ld(din["enc"][j * P:(j + 1) * P, :], [P, H], f"enc{j}") for j in range(KT)]
    wo = [ld(din["wo"][k * P:(k + 1) * P, :], [P, H], f"wo{k}") for k in range(2 * KT)]
    bor = ld(din["bo"][:], [1, H], "bor")
    eye64 = ld(din["eye64"][:], [T, T], "eye64")
    ones64 = pers.tile([1, T], f32, tag="ones64")
    nc.vector.memset(ones64[:], 1.0)

    if PROBES:
        pr = ld(din["probe"][:], [P, T], "probe")
        ps = pers.tile([P, T], f32, tag="probe_sin")
        nc.scalar.activation(ps[:], pr[:], AF.Sin)
        nc.sync.dma_start(out=dout["probe_sin"][:], in_=ps[:])
        pm = pers.tile([P, T], f32, tag="probe_mod")
        ki = pers.tile([P, T], mybir.dt.int32, tag="probe_ki")
        nc.vector.tensor_scalar(ki[:], pr[:], float(1.0 / (2 * np.pi)), None, ALU.mult)
        nc.vector.tensor_copy(pm[:], ki[:])
        nc.sync.dma_start(out=dout["probe_mod"][:], in_=pm[:])

    # coverage: enc = tanh(enc_raw + cov (x) wcov) in both layouts
    for j in range(KT):  # [S,H] layout: outer[s,h] = cov[s]*wcov[h]
        op = psT.tile([P, H], f32, tag="pt")
        nc.tensor.matmul(op[:], covr[0:1, j * P:(j + 1) * P], wcovr[0:1, :],
                         start=True, stop=True)
        nc.vector.tensor_add(enc[j][:], enc[j][:], op[:])
        nc.scalar.activation(enc[j][:], enc[j][:], AF.Tanh)
    for i in range(KT):  # [H,S] layout
        op = psT.tile([P, S], f32, tag="pt")
        nc.tensor.matmul(op[:], wcovr[0:1, i * P:(i + 1) * P], covr[0:1, :],
                         start=True, stop=True)
        nc.vector.tensor_add(encT[i], encT[i], op[:])
        nc.scalar.activation(encT[i], encT[i], AF.Tanh)

    # a2T[hout, s] = sum_hin Wc[hin,hout] * encT[hin,s]   (one [128, KT*S] tile)
    a2T = pers.tile([P, KT * S], f32, tag="a2T")
    for m in range(KT):
        pm2 = psT.tile([P, S], f32, tag="pt")
        for k in range(KT):
            nc.tensor.matmul(pm2[:], wc[k][:, m * P:(m + 1) * P], encT[k],
                             start=(k == 0), stop=(k == KT - 1))
        nc.scalar.copy(a2T[:, m * S:(m + 1) * S], pm2[:])

    # a1T[hout, t] = sum_hin Wq[hin,hout] * decT[hin,t] + bq[hout]
    a1T = pers.tile([P, KT * T], f32, tag="a1T")
    for m in range(KT):
        pm1 = psSm.tile([P, T], f32, tag="ps")
        for k in range(KT):
            nc.tensor.matmul(pm1[:], wq[k][:, m * P:(m + 1) * P], decT[k][:],
                             start=(k == 0), stop=False)
        nc.tensor.matmul(pm1[:], bqr[0:1, m * P:(m + 1) * P], ones64[0:1, :],
                         start=False, stop=True)
        nc.scalar.copy(a1T[:, m * T:(m + 1) * T], pm1[:])

    scores = psOut.tile([T, S], f32, tag="out512")

    if variant == "exact":
        vwinr = []
        for k in range(KT):
            vr = pers.tile([P, 127], f32r, tag=f"vwinr{k}")
            nc.vector.tensor_copy(vr[:].bitcast(f32r), vwin[k][:])
            vwinr.append(vr)
        NG = T // TG
        first = True
        for g in range(NG):
            bb = pb4.tile([P, KT * TG * S], f32, tag="bb")
            for k in range(KT):
                for tt in range(TG):
                    t_ = g * TG + tt
                    seg = bb[:, (k * TG + tt) * S:(k * TG + tt + 1) * S]
                    nc.vector.tensor_scalar_add(
                        seg, a2T[:, k * S:(k + 1) * S],
                        a1T[:, k * T + t_:k * T + t_ + 1])
            tb = big.tile([P, KT * TG * S], f32r, tag="tb")
            nc.scalar.activation(tb[:], bb[:], AF.Tanh)
            for k in range(KT):
                for tt in range(TG):
                    t_ = g * TG + tt
                    seg = tb[:, (k * TG + tt) * S:(k * TG + tt + 1) * S]
                    last = (g == NG - 1 and k == KT - 1 and tt == TG - 1)
                    nc.tensor.matmul(scores[:], r(vwinr[k][:, 63 - t_:127 - t_]), seg,
                                     start=first, stop=last)
                    first = False
    else:  # fourier
        MF = len(FREQ_W)
        OFF = float(12 * np.pi)  # multiple of 2*pi; makes mod input positive
        HALF_PI = float(np.pi / 2)
        negpi = pers.tile([P, 1], f32, tag="negpi")
        nc.vector.memset(negpi[:], float(-np.pi))
        # a1-side features, batched over all frequencies: [128, MF*KT*T]
        FW = KT * T
        y1a = pb2.tile([P, MF * FW], f32, tag="tmp1")
        for i in range(MF):
            nc.vector.tensor_scalar(y1a[:, i * FW:(i + 1) * FW], a1T[:],
                                    float(FREQ_W[i]), OFF, ALU.mult, ALU.add)
        u1s = pb2.tile([P, MF * FW], f32, tag="tmp1")
        s1a = pers.tile([P, MF * FW], f32, tag="s1a")
        nc.vector.tensor_scalar(u1s[:], y1a[:], TWO_PI, None, ALU.mod)
        nc.scalar.activation(s1a[:], u1s[:], AF.Sin, bias=negpi[:])
        u1c = pb2.tile([P, MF * FW], f32, tag="tmp1")
        c1a = pers.tile([P, MF * FW], f32, tag="c1a")
        nc.vector.tensor_scalar(u1c[:], y1a[:], HALF_PI, TWO_PI, ALU.add, ALU.mod)
        nc.scalar.activation(c1a[:], u1c[:], AF.Sin, bias=negpi[:])
        for i in range(MF):
            wi = float(FREQ_W[i])
            bi = float(FREQ_B[i])
            # a2-side features (sign-flipped: sin(u - pi) = -sin(arg))
            y2 = pb2.tile([P, KT * S], f32, tag="y2")
            nc.vector.tensor_scalar(y2[:], a2T[:], wi, OFF, ALU.mult, ALU.add)
            u2s = pb4.tile([P, KT * S], f32, tag="u2")
            nc.vector.tensor_scalar(u2s[:], y2[:], TWO_PI, None, ALU.mod)
            s2 = pb4.tile([P, KT * S], f32, tag="f2")
            nc.scalar.activation(s2[:].bitcast(f32r), u2s[:], AF.Sin, bias=negpi[:])
            u2c = pb4.tile([P, KT * S], f32, tag="u2")
            nc.vector.tensor_scalar(u2c[:], y2[:], HALF_PI, TWO_PI, ALU.add, ALU.mod)
            c2 = pb4.tile([P, KT * S], f32, tag="f2")
            nc.scalar.activation(c2[:].bitcast(f32r), u2c[:], AF.Sin, bias=negpi[:])
            # scale a1 features by v[h]*b_i per hout chunk
            ws1 = pb4.tile([P, FW], f32, tag="wf1")
            wc1 = pb4.tile([P, FW], f32, tag="wf1")
            for k in range(KT):
                sl = slice(k * T, (k + 1) * T)
                nc.vector.tensor_scalar(ws1[:, sl].bitcast(f32r),
                                        s1a[:, i * FW + k * T:i * FW + (k + 1) * T],
                                        vwin[k][:, 63:64], bi, ALU.mult, ALU.mult)
                nc.vector.tensor_scalar(wc1[:, sl].bitcast(f32r),
                                        c1a[:, i * FW + k * T:i * FW + (k + 1) * T],
                                        vwin[k][:, 63:64], bi, ALU.mult, ALU.mult)
            for k in range(KT):
                nc.tensor.matmul(scores[:], r(ws1[:, k * T:(k + 1) * T]),
                                 r(c2[:, k * S:(k + 1) * S]),
                                 start=(i == 0 and k == 0), stop=False)
                nc.tensor.matmul(scores[:], r(wc1[:, k * T:(k + 1) * T]),
                                 r(s2[:, k * S:(k + 1) * S]),
                                 start=False, stop=(i == MF - 1 and k == KT - 1))

    # softmax over s (free dim)
    negmax = pers.tile([T, 1], f32, tag="negmax")
    nc.vector.tensor_reduce(negmax[:], scores[:], axis=AX.X, op=ALU.max, negate=True)
    align_sb = pers.tile([T, S], f32, tag="align_sb")
    sums = pers.tile([T, 1], f32, tag="sums")
    nc.scalar.activation(align_sb[:], scores[:], AF.Exp, bias=negmax[:],
                         accum_out=sums[:])
    recips = pers.tile([T, 1], f32, tag="recips")
    nc.vector.reciprocal(recips[:], sums[:])
    nc.vector.tensor_scalar_mul(align_sb[:], align_sb[:], recips[:])
    nc.sync.dma_start(out=dout["align"][:], in_=align_sb[:])

    # alignT via PE transpose
    alignT = []
    for j in range(KT):
        pt = psSm.tile([P, T], f32, tag="ps")
        nc.tensor.transpose(pt[:], align_sb[:, j * P:(j + 1) * P], eye64[:])
        at = pers.tile([P, T], f32, tag=f"alignT{j}")
        nc.vector.tensor_copy(at[:], pt[:])
        alignT.append(at)

    # cT[h, t] = sum_s enc[s,h] * alignT[s,t]
    cT = []
    for m in range(KT):
        pc = psSm.tile([P, T], f32, tag="ps")
        for j in range(KT):
            nc.tensor.matmul(pc[:], enc[j][:, m * P:(m + 1) * P], alignT[j][:],
                             start=(j == 0), stop=(j == KT - 1))
        ct = pers.tile([P, T], f32, tag=f"cT{m}")
        nc.vector.tensor_copy(ct[:], pc[:])
        cT.append(ct)

    # attn_h = [c, dec] @ Wo + bo
    pa = psOut.tile([T, H], f32, tag="out512")
    for k in range(KT):
        nc.tensor.matmul(pa[:], cT[k][:], wo[k][:], start=(k == 0), stop=False)
    for k in range(KT):
        nc.tensor.matmul(pa[:], decT[k][:], wo[KT + k][:], start=False, stop=False)
    nc.tensor.matmul(pa[:], ones64[0:1, :], bor[0:1, :], start=False, stop=True)
    attn_sb = pers.tile([T, H], f32, tag="attn_sb")
    nc.vector.tensor_copy(attn_sb[:], pa[:])
    nc.sync.dma_start(out=dout["attn_h"][:], in_=attn_sb[:])


def build(variant=None):
    variant = variant or VARIANT
    if variant in _BUILT:
        return _BUILT[variant]
    from contextlib import ExitStack

    import concourse.bacc as bacc
    import concourse.mybir as mybir
    import concourse.tile as tile

    f32 = mybir.dt.float32
    nc = bacc.Bacc("TRN2", target_bir_lowering=False, debug=False)
    in_specs = [
        ("decT", [H, T]), ("enc", [S, H]), ("encT", [H, S]),
        ("wq", [H, H]), ("wc", [H, H]), ("wo", [2 * H, H]),
        ("vwin", [KT, P, 127]), ("cov", [1, S]), ("wcov", [1, H]),
        ("bq", [1, H]), ("bo", [1, H]), ("eye64", [T, T]),
    ]
    out_specs = [("attn_h", [T, H]), ("align", [T, S])]
    if PROBES:
        in_specs.append(("probe", [P, T]))
        out_specs += [("probe_sin", [P, T]), ("probe_mod", [P, T])]
    din = {n: nc.declare_dram_parameter(n, s, f32, isOutput=False) for n, s in in_specs}
    dout = {n: nc.declare_dram_parameter(n, s, f32, isOutput=True) for n, s in out_specs}
    with ExitStack() as ctx:
        tc = ctx.enter_context(tile.TileContext(nc))
        _emit(nc, tc, ctx, din, dout, variant)
    nc.compile()
    _BUILT[variant] = nc
    return nc


def prep_core_inputs(inputs):
    """Host-side shard: per-core input dicts (core b <- batch element b)."""
    dec = np.asarray(inputs["attn_dec_state"], np.float32)  # [T,B,H]
    encr = np.asarray(inputs["attn_enc_state"], np.float32)  # [S,B,H]
    cov = np.asarray(inputs["attn_coverage"], np.float32)  # [B,S]
    Wq = np.ascontiguousarray(np.asarray(inputs["Wq"], np.float32))
    Wc = np.ascontiguousarray(np.asarray(inputs["Wc"], np.float32))
    Wo = np.ascontiguousarray(np.asarray(inputs["Wo"], np.float32))
    v = np.asarray(inputs["v"], np.float32)
    bq = np.asarray(inputs["bq"], np.float32)[None, :]
    bo = np.asarray(inputs["bo"], np.float32)[None, :]
    wcov = np.asarray(inputs["wcov"], np.float32)[None, :]
    vwin = np.zeros((KT, P, 127), np.float32)
    for k in range(KT):
        vwin[k, :, 63] = v[k * P:(k + 1) * P]
    eye64 = np.eye(T, dtype=np.float32)
    shared = dict(wq=Wq, wc=Wc, wo=Wo, vwin=vwin, wcov=wcov, bq=bq, bo=bo,
                  eye64=eye64)
    if PROBES:
        shared["probe"] = np.linspace(-16, 16, P * T).astype(np.float32).reshape(P, T)
    maps = []
    for b in range(B):
        e = np.ascontiguousarray(encr[:, b, :])
        maps.append(dict(
            decT=np.ascontiguousarray(dec[:, b, :].T),
            enc=e,
            encT=np.ascontiguousarray(e.T),
            cov=np.ascontiguousarray(cov[b][None, :]),
            **shared,
        ))
    return maps


def kernel(**inputs):
    global LAST_RESULT
    nc = build()
    in_maps = prep_core_inputs(inputs)
    from concourse.bass_utils import run_bass_kernel_spmd

    trace = os.environ.get("ATTN_TRACE", "0") == "1"
    res = run_bass_kernel_spmd(nc, in_maps, list(range(B)), trace=trace)
    LAST_RESULT = res
    attn_h = np.stack([res.results[i]["attn_h"] for i in range(B)], axis=1)
    align = np.stack([res.results[i]["align"] for i in range(B)], axis=1)
    return attn_h, align


# revision 18
# speedup vs baseline: 9956.9817x; 1.0571x over previous
"""Trainium2 Bass kernel: Bahdanau (additive) attention with coverage.

Reference computation (per batch element b, data-parallel over B=8 cores):
    enc   = tanh(enc_raw + cov[:,None]*wcov)            [S,H]
    a1    = dec @ Wq + bq                               [T,H]
    a2    = enc @ Wc                                    [S,H]
    scores[t,s] = sum_h v[h] * tanh(a1[t,h] + a2[s,h])  [T,S]
    align = softmax(scores, -1)                         [T,S]
    c     = align @ enc                                 [T,H]
    attn_h = [c, dec] @ Wo + bo                         [T,H]
Outputs: attn_h -> [T,B,H], align -> [T,B,S].

Two device variants (ATTN_VARIANT env, default "exact"):
  exact   : big-buffer DVE outer-sum (a1+a2, tensor_scalar per-partition bias)
            + one big ACT tanh per t-group (f32r output) + PE v-dot into a
            single [64,512] PSUM scores tile via a sliding-window masked-v
            lhsT (v at column 63 of a [128,127] window; offset 63-t puts it
            at output row t). ACT-bound at ~175us/core. Verified on HW.
  fourier : tanh(z) ~ sum_i b_i sin(w_i z), split by angle addition into PE
            matmuls over sin/cos feature maps. CoreSim-correct but NOT HW
            viable: DVE tensor_scalar 'mod' is ISA-invalid and ACT Sin is
            only accurate for |x| <= 3.795, so the required range reduction
            costs more DVE time than the ACT it saves. Kept for reference.
"""

import os

import numpy as np

T, B, S, H = 64, 8, 512, 512
P = 128
KT = H // P  # 4 partition tiles of H

VARIANT = os.environ.get("ATTN_VARIANT", "exact")  # "exact" | "fourier"
PROBES = os.environ.get("ATTN_PROBES", "0") == "1"
TG = 2  # t-group size (exact variant)
M_F = int(os.environ.get("ATTN_M", "16"))  # number of sine harmonics (fourier)
L_F = 11.0  # half-period of the sine basis
TWO_PI = float(2 * np.pi)

FREQ_W = [0.2837544416, 0.8559440962, 1.4398952546, 2.0384750571,
          2.6518142442, 3.2791112395, 3.9194354832, 4.5716932015,
          5.2332940231, 5.8897412933]
FREQ_B = [1.2338354998, 0.3228807754, 0.1244921527, 0.0493901935,
          0.0192673255, 0.0073474932, 0.0027404072, 0.001000557,
          0.0003570621, 0.0001197961]

_BUILT = {}
LAST_RESULT = None


def _sine_coefs():
    z = np.linspace(-9.0, 9.0, 4001)
    A = np.sin(np.pi / L_F * np.outer(z, np.arange(1, M_F + 1)))
    b, *_ = np.linalg.lstsq(A, np.tanh(z), rcond=None)
    return b


def _emit(nc, tc, ctx, din, dout, variant):
    import concourse.mybir as mybir

    f32 = mybir.dt.float32
    AF = mybir.ActivationFunctionType
    ALU = mybir.AluOpType
    AX = mybir.AxisListType

    f32r = mybir.dt.float32r

    def r(ap):
        return ap.bitcast(f32r)

    pers = ctx.enter_context(tc.tile_pool(name="pers", bufs=1))
    big = ctx.enter_context(tc.tile_pool(name="big", bufs=3))
    pb2 = ctx.enter_context(tc.tile_pool(name="pb2", bufs=2))
    pb4 = ctx.enter_context(tc.tile_pool(name="pb4", bufs=4))
    psT = ctx.enter_context(tc.tile_pool(name="psT", bufs=2, space="PSUM"))
    psSm = ctx.enter_context(tc.tile_pool(name="psSm", bufs=2, space="PSUM"))
    psOut = ctx.enter_context(tc.tile_pool(name="psOut", bufs=1, space="PSUM"))

    def ld(dram_ap, shape, tag):
        t = pers.tile(shape, f32, tag=tag)
        nc.sync.dma_start(out=t[:], in_=dram_ap)
        return t

    def ld_merged(pool, dram, n_chunks, chunk_f, tag):
        t = pool.tile([P, n_chunks * chunk_f], f32, tag=tag)
        nc.sync.dma_start(
            out=t[:].rearrange("p (c f) -> p c f", c=n_chunks),
            in_=dram[:].rearrange("(c p) f -> p c f", p=P))
        return t

    covr = ld(din["cov"][:], [1, S], "covr")
    wcovr = ld(din["wcov"][:], [1, H], "wcovr")
    if variant == "fourier":
        encT_all = pb4.tile([P, KT * S], f32, tag="u2", name="encT_all")
    else:
        encT_all = pb2.tile([P, KT * S], f32, tag="bb", name="encT_all")
    nc.sync.dma_start(
        out=encT_all[:].rearrange("p (c f) -> p c f", c=KT),
        in_=din["encT"][:].rearrange("(c p) f -> p c f", p=P))
    encT = [encT_all[:, i * S:(i + 1) * S] for i in range(KT)]
    wc_all = ld_merged(pers, din["wc"], KT, H, "wc")
    wc = [wc_all[:, k * H:(k + 1) * H] for k in range(KT)]
    decT_all = ld_merged(pers, din["decT"], KT, T, "decT")
    decT = [decT_all[:, k * T:(k + 1) * T] for k in range(KT)]
    wq_all = ld_merged(pers, din["wq"], KT, H, "wq")
    wq = [wq_all[:, k * H:(k + 1) * H] for k in range(KT)]
    vwin_all = ld_merged(pers, din["vwin"].reshape([KT * P, 127]), KT, 127, "vwin")
    vwin = [vwin_all[:, k * 127:(k + 1) * 127] for k in range(KT)]
    bqr = ld(din["bq"][:], [1, H], "bqr")
    wo_all = ld_merged(pers, din["wo"], 2 * KT, H, "wo")
    wo = [wo_all[:, k * H:(k + 1) * H] for k in range(2 * KT)]
    bor = ld(din["bo"][:], [1, H], "bor")
    eye64 = ld(din["eye64"][:], [T, T], "eye64")
    ones64 = pers.tile([1, T], f32, tag="ones64")
    nc.vector.memset(ones64[:], 1.0)

    if PROBES:
        pr = ld(din["probe"][:], [P, T], "probe")
        ps = pers.tile([P, T], f32, tag="probe_sin")
        nc.scalar.activation(ps[:], pr[:], AF.Sin)
        nc.sync.dma_start(out=dout["probe_sin"][:], in_=ps[:])
        pm = pers.tile([P, T], f32, tag="probe_mod")
        ki = pers.tile([P, T], mybir.dt.int32, tag="probe_ki")
        nc.vector.tensor_scalar(ki[:], pr[:], float(1.0 / (2 * np.pi)), None, ALU.mult)
        nc.vector.tensor_copy(pm[:], ki[:])
        nc.sync.dma_start(out=dout["probe_mod"][:], in_=pm[:])

    # coverage: enc = tanh(enc_raw + cov (x) wcov) in both layouts.
    # The [S,H]-layout copy is only needed by the epilogue c-matmul, so its
    # DMA + coverage are emitted after the main loop (emit_enc below).
    def emit_enc():
        enc_all = (big.tile([P, KT * H], f32, tag="tb", name="enc_all")
                   if variant == "exact"
                   else pers.tile([P, KT * H], f32, tag="enc", name="enc_all"))
        nc.sync.dma_start(
            out=enc_all[:].rearrange("p (c f) -> p c f", c=KT),
            in_=din["enc"][:].rearrange("(c p) f -> p c f", p=P))
        enc = [enc_all[:, j * H:(j + 1) * H] for j in range(KT)]
        for j in range(KT):  # [S,H] layout: outer[s,h] = cov[s]*wcov[h]
            op = psT.tile([P, H], f32, tag="pt")
            nc.tensor.matmul(op[:], covr[0:1, j * P:(j + 1) * P], wcovr[0:1, :],
                             start=True, stop=True)
            nc.vector.tensor_add(enc[j], enc[j], op[:])
            nc.scalar.activation(enc[j], enc[j], AF.Tanh)
        return enc
    for i in range(KT):  # [H,S] layout
        op = psT.tile([P, S], f32, tag="pt")
        nc.tensor.matmul(op[:], wcovr[0:1, i * P:(i + 1) * P], covr[0:1, :],
                         start=True, stop=True)
        nc.vector.tensor_add(encT[i], encT[i], op[:])
        nc.scalar.activation(encT[i], encT[i], AF.Tanh)

    # a2T[hout, s] = sum_hin Wc[hin,hout] * encT[hin,s]   (one [128, KT*S] tile)
    a2T = pers.tile([P, KT * S], f32, tag="a2T")
    for m in range(KT):
        pm2 = psT.tile([P, S], f32, tag="pt")
        for k in range(KT):
            nc.tensor.matmul(pm2[:], wc[k][:, m * P:(m + 1) * P], encT[k],
                             start=(k == 0), stop=(k == KT - 1))
        nc.scalar.copy(a2T[:, m * S:(m + 1) * S], pm2[:])

    # a1T[hout, t] = sum_hin Wq[hin,hout] * decT[hin,t] + bq[hout]
    a1T = pers.tile([P, KT * T], f32, tag="a1T")
    for m in range(KT):
        pm1 = psSm.tile([P, T], f32, tag="ps")
        for k in range(KT):
            nc.tensor.matmul(pm1[:], wq[k][:, m * P:(m + 1) * P], decT[k][:],
                             start=(k == 0), stop=False)
        nc.tensor.matmul(pm1[:], bqr[0:1, m * P:(m + 1) * P], ones64[0:1, :],
                         start=False, stop=True)
        nc.scalar.copy(a1T[:, m * T:(m + 1) * T], pm1[:])

    scores = psOut.tile([T, S], f32, tag="out512")

    if variant == "exact":
        vwinr = []
        for k in range(KT):
            vr = pers.tile([P, 127], f32r, tag=f"vwinr{k}")
            nc.vector.tensor_copy(vr[:].bitcast(f32r), vwin[k][:])
            vwinr.append(vr)
        NG = T // TG
        first = True
        for g in range(NG):
            bb = pb2.tile([P, KT * TG * S], f32, tag="bb")
            for k in range(KT):
                for tt in range(TG):
                    t_ = g * TG + tt
                    seg = bb[:, (k * TG + tt) * S:(k * TG + tt + 1) * S]
                    nc.vector.tensor_scalar_add(
                        seg, a2T[:, k * S:(k + 1) * S],
                        a1T[:, k * T + t_:k * T + t_ + 1])
            tb = big.tile([P, KT * TG * S], f32r, tag="tb")
            nc.scalar.activation(tb[:], bb[:], AF.Tanh)
            for k in range(KT):
                for tt in range(TG):
                    t_ = g * TG + tt
                    seg = tb[:, (k * TG + tt) * S:(k * TG + tt + 1) * S]
                    last = (g == NG - 1 and k == KT - 1 and tt == TG - 1)
                    nc.tensor.matmul(scores[:], r(vwinr[k][:, 63 - t_:127 - t_]), seg,
                                     start=first, stop=last)
                    first = False
    else:  # fourier
        MF = len(FREQ_W)
        OFF = float(12 * np.pi)  # multiple of 2*pi; makes mod input positive
        HALF_PI = float(np.pi / 2)
        negpi = pers.tile([P, 1], f32, tag="negpi")
        nc.vector.memset(negpi[:], float(-np.pi))
        # a1-side features, batched over all frequencies: [128, MF*KT*T]
        FW = KT * T
        y1a = pb2.tile([P, MF * FW], f32, tag="tmp1")
        for i in range(MF):
            nc.vector.tensor_scalar(y1a[:, i * FW:(i + 1) * FW], a1T[:],
                                    float(FREQ_W[i]), OFF, ALU.mult, ALU.add)
        u1s = pb2.tile([P, MF * FW], f32, tag="tmp1")
        s1a = pers.tile([P, MF * FW], f32, tag="s1a")
        nc.vector.tensor_scalar(u1s[:], y1a[:], TWO_PI, None, ALU.mod)
        nc.scalar.activation(s1a[:], u1s[:], AF.Sin, bias=negpi[:])
        u1c = pb2.tile([P, MF * FW], f32, tag="tmp1")
        c1a = pers.tile([P, MF * FW], f32, tag="c1a")
        nc.vector.tensor_scalar(u1c[:], y1a[:], HALF_PI, TWO_PI, ALU.add, ALU.mod)
        nc.scalar.activation(c1a[:], u1c[:], AF.Sin, bias=negpi[:])
        for i in range(MF):
            wi = float(FREQ_W[i])
            bi = float(FREQ_B[i])
            # a2-side features (sign-flipped: sin(u - pi) = -sin(arg))
            y2 = pb2.tile([P, KT * S], f32, tag="y2")
            nc.vector.tensor_scalar(y2[:], a2T[:], wi, OFF, ALU.mult, ALU.add)
            u2s = pb4.tile([P, KT * S], f32, tag="u2")
            nc.vector.tensor_scalar(u2s[:], y2[:], TWO_PI, None, ALU.mod)
            s2 = pb4.tile([P, KT * S], f32, tag="f2")
            nc.scalar.activation(s2[:].bitcast(f32r), u2s[:], AF.Sin, bias=negpi[:])
            u2c = pb4.tile([P, KT * S], f32, tag="u2")
            nc.vector.tensor_scalar(u2c[:], y2[:], HALF_PI, TWO_PI, ALU.add, ALU.mod)
            c2 = pb4.tile([P, KT * S], f32, tag="f2")
            nc.scalar.activation(c2[:].bitcast(f32r), u2c[:], AF.Sin, bias=negpi[:])
            # scale a1 features by v[h]*b_i per hout chunk
            ws1 = pb4.tile([P, FW], f32, tag="wf1")
            wc1 = pb4.tile([P, FW], f32, tag="wf1")
            for k in range(KT):
                sl = slice(k * T, (k + 1) * T)
                nc.vector.tensor_scalar(ws1[:, sl].bitcast(f32r),
                                        s1a[:, i * FW + k * T:i * FW + (k + 1) * T],
                                        vwin[k][:, 63:64], bi, ALU.mult, ALU.mult)
                nc.vector.tensor_scalar(wc1[:, sl].bitcast(f32r),
                                        c1a[:, i * FW + k * T:i * FW + (k + 1) * T],
                                        vwin[k][:, 63:64], bi, ALU.mult, ALU.mult)
            for k in range(KT):
                nc.tensor.matmul(scores[:], r(ws1[:, k * T:(k + 1) * T]),
                                 r(c2[:, k * S:(k + 1) * S]),
                                 start=(i == 0 and k == 0), stop=False)
                nc.tensor.matmul(scores[:], r(wc1[:, k * T:(k + 1) * T]),
                                 r(s2[:, k * S:(k + 1) * S]),
                                 start=False, stop=(i == MF - 1 and k == KT - 1))

    enc = emit_enc()

    # softmax over s (free dim)
    negmax = pers.tile([T, 1], f32, tag="negmax")
    nc.vector.tensor_reduce(negmax[:], scores[:], axis=AX.X, op=ALU.max, negate=True)
    align_sb = pers.tile([T, S], f32, tag="align_sb")
    sums = pers.tile([T, 1], f32, tag="sums")
    nc.scalar.activation(align_sb[:], scores[:], AF.Exp, bias=negmax[:],
                         accum_out=sums[:])
    recips = pers.tile([T, 1], f32, tag="recips")
    nc.vector.reciprocal(recips[:], sums[:])
    nc.vector.tensor_scalar_mul(align_sb[:], align_sb[:], recips[:])
    nc.sync.dma_start(out=dout["align"][:], in_=align_sb[:])

    # alignT via PE transpose
    alignT = []
    for j in range(KT):
        pt = psSm.tile([P, T], f32, tag="ps")
        nc.tensor.transpose(pt[:], align_sb[:, j * P:(j + 1) * P], eye64[:])
        at = pers.tile([P, T], f32, tag=f"alignT{j}")
        nc.vector.tensor_copy(at[:], pt[:])
        alignT.append(at)

    # cT[h, t] = sum_s enc[s,h] * alignT[s,t]
    cT = []
    for m in range(KT):
        pc = psSm.tile([P, T], f32, tag="ps")
        for j in range(KT):
            nc.tensor.matmul(pc[:], enc[j][:, m * P:(m + 1) * P], alignT[j][:],
                             start=(j == 0), stop=(j == KT - 1))
        ct = pers.tile([P, T], f32, tag=f"cT{m}")
        nc.vector.tensor_copy(ct[:], pc[:])
        cT.append(ct)

    # attn_h = [c, dec] @ Wo + bo
    pa = psOut.tile([T, H], f32, tag="out512")
    for k in range(KT):
        nc.tensor.matmul(pa[:], cT[k][:], wo[k][:], start=(k == 0), stop=False)
    for k in range(KT):
        nc.tensor.matmul(pa[:], decT[k][:], wo[KT + k][:], start=False, stop=False)
    nc.tensor.matmul(pa[:], ones64[0:1, :], bor[0:1, :], start=False, stop=True)
    attn_sb = pers.tile([T, H], f32, tag="attn_sb")
    nc.vector.tensor_copy(attn_sb[:], pa[:])
    nc.sync.dma_start(out=dout["attn_h"][:], in_=attn_sb[:])


def build(variant=None):
    variant = variant or VARIANT
    if variant in _BUILT:
        return _BUILT[variant]
    from contextlib import ExitStack

    import concourse.bacc as bacc
    import concourse.mybir as mybir
    import concourse.tile as tile

    f32 = mybir.dt.float32
    nc = bacc.Bacc("TRN2", target_bir_lowering=False, debug=False)
    in_specs = [
        ("decT", [H, T]), ("enc", [S, H]), ("encT", [H, S]),
        ("wq", [H, H]), ("wc", [H, H]), ("wo", [2 * H, H]),
        ("vwin", [KT, P, 127]), ("cov", [1, S]), ("wcov", [1, H]),
        ("bq", [1, H]), ("bo", [1, H]), ("eye64", [T, T]),
    ]
    out_specs = [("attn_h", [T, H]), ("align", [T, S])]
    if PROBES:
        in_specs.append(("probe", [P, T]))
        out_specs += [("probe_sin", [P, T]), ("probe_mod", [P, T])]
    din = {n: nc.declare_dram_parameter(n, s, f32, isOutput=False) for n, s in in_specs}
    dout = {n: nc.declare_dram_parameter(n, s, f32, isOutput=True) for n, s in out_specs}
    with ExitStack() as ctx:
        tc = ctx.enter_context(tile.TileContext(nc))
        _emit(nc, tc, ctx, din, dout, variant)
    nc.compile()
    _BUILT[variant] = nc
    return nc


def prep_core_inputs(inputs):
    """Host-side shard: per-core input dicts (core b <- batch element b)."""
    dec = np.asarray(inputs["attn_dec_state"], np.float32)  # [T,B,H]
    encr = np.asarray(inputs["attn_enc_state"], np.float32)  # [S,B,H]
    cov = np.asarray(inputs["attn_coverage"], np.float32)  # [B,S]
    Wq = np.ascontiguousarray(np.asarray(inputs["Wq"], np.float32))
    Wc = np.ascontiguousarray(np.asarray(inputs["Wc"], np.float32))
    Wo = np.ascontiguousarray(np.asarray(inputs["Wo"], np.float32))
    v = np.asarray(inputs["v"], np.float32)
    bq = np.asarray(inputs["bq"], np.float32)[None, :]
    bo = np.asarray(inputs["bo"], np.float32)[None, :]
    wcov = np.asarray(inputs["wcov"], np.float32)[None, :]
    vwin = np.zeros((KT, P, 127), np.float32)
    for k in range(KT):
        vwin[k, :, 63] = v[k * P:(k + 1) * P]
    eye64 = np.eye(T, dtype=np.float32)
    shared = dict(wq=Wq, wc=Wc, wo=Wo, vwin=vwin, wcov=wcov, bq=bq, bo=bo,
                  eye64=eye64)
    if PROBES:
        shared["probe"] = np.linspace(-16, 16, P * T).astype(np.float32).reshape(P, T)
    maps = []
    for b in range(B):
        e = np.ascontiguousarray(encr[:, b, :])
        maps.append(dict(
            decT=np.ascontiguousarray(dec[:, b, :].T),
            enc=e,
            encT=np.ascontiguousarray(e.T),
            cov=np.ascontiguousarray(cov[b][None, :]),
            **shared,
        ))
    return maps


def kernel(**inputs):
    global LAST_RESULT
    nc = build()
    in_maps = prep_core_inputs(inputs)
    from concourse.bass_utils import run_bass_kernel_spmd

    trace = os.environ.get("ATTN_TRACE", "0") == "1"
    res = run_bass_kernel_spmd(nc, in_maps, list(range(B)), trace=trace)
    LAST_RESULT = res
    attn_h = np.stack([res.results[i]["attn_h"] for i in range(B)], axis=1)
    align = np.stack([res.results[i]["align"] for i in range(B)], axis=1)
    return attn_h, align


# revision 23
# speedup vs baseline: 10374.1128x; 1.0419x over previous
"""Trainium2 Bass kernel: Bahdanau (additive) attention with coverage.

Reference computation (per batch element b, data-parallel over B=8 cores):
    enc   = tanh(enc_raw + cov[:,None]*wcov)            [S,H]
    a1    = dec @ Wq + bq                               [T,H]
    a2    = enc @ Wc                                    [S,H]
    scores[t,s] = sum_h v[h] * tanh(a1[t,h] + a2[s,h])  [T,S]
    align = softmax(scores, -1)                         [T,S]
    c     = align @ enc                                 [T,H]
    attn_h = [c, dec] @ Wo + bo                         [T,H]
Outputs: attn_h -> [T,B,H], align -> [T,B,S].

Two device variants (ATTN_VARIANT env, default "exact"):
  exact   : big-buffer DVE outer-sum (a1+a2, tensor_scalar per-partition bias)
            + one big ACT tanh per t-group (f32r output) + PE v-dot into a
            single [64,512] PSUM scores tile via a sliding-window masked-v
            lhsT (v at column 63 of a [128,127] window; offset 63-t puts it
            at output row t). ACT-bound at ~175us/core. Verified on HW.
  fourier : tanh(z) ~ sum_i b_i sin(w_i z), split by angle addition into PE
            matmuls over sin/cos feature maps. CoreSim-correct but NOT HW
            viable: DVE tensor_scalar 'mod' is ISA-invalid and ACT Sin is
            only accurate for |x| <= 3.795, so the required range reduction
            costs more DVE time than the ACT it saves. Kept for reference.
"""

import os

import numpy as np

T, B, S, H = 64, 8, 512, 512
P = 128
KT = H // P  # 4 partition tiles of H

VARIANT = os.environ.get("ATTN_VARIANT", "exact")  # "exact" | "fourier"
PROBES = os.environ.get("ATTN_PROBES", "0") == "1"
TG = 2  # t-group size (exact variant)
M_F = int(os.environ.get("ATTN_M", "16"))  # number of sine harmonics (fourier)
L_F = 11.0  # half-period of the sine basis
TWO_PI = float(2 * np.pi)

FREQ_W = [0.2837544416, 0.8559440962, 1.4398952546, 2.0384750571,
          2.6518142442, 3.2791112395, 3.9194354832, 4.5716932015,
          5.2332940231, 5.8897412933]
FREQ_B = [1.2338354998, 0.3228807754, 0.1244921527, 0.0493901935,
          0.0192673255, 0.0073474932, 0.0027404072, 0.001000557,
          0.0003570621, 0.0001197961]

_BUILT = {}
LAST_RESULT = None


def _sine_coefs():
    z = np.linspace(-9.0, 9.0, 4001)
    A = np.sin(np.pi / L_F * np.outer(z, np.arange(1, M_F + 1)))
    b, *_ = np.linalg.lstsq(A, np.tanh(z), rcond=None)
    return b


def _emit(nc, tc, ctx, din, dout, variant):
    import concourse.mybir as mybir

    f32 = mybir.dt.float32
    AF = mybir.ActivationFunctionType
    ALU = mybir.AluOpType
    AX = mybir.AxisListType

    f32r = mybir.dt.float32r

    def r(ap):
        return ap.bitcast(f32r)

    pers = ctx.enter_context(tc.tile_pool(name="pers", bufs=1))
    big = ctx.enter_context(tc.tile_pool(name="big", bufs=3))
    pb2 = ctx.enter_context(tc.tile_pool(name="pb2", bufs=2))
    pb4 = ctx.enter_context(tc.tile_pool(name="pb4", bufs=4))
    psT = ctx.enter_context(tc.tile_pool(name="psT", bufs=2, space="PSUM"))
    psSm = ctx.enter_context(tc.tile_pool(name="psSm", bufs=2, space="PSUM"))
    psOut = ctx.enter_context(tc.tile_pool(name="psOut", bufs=1, space="PSUM"))

    def ld(dram_ap, shape, tag):
        t = pers.tile(shape, f32, tag=tag)
        nc.sync.dma_start(out=t[:], in_=dram_ap)
        return t

    def ld_merged(pool, dram, n_chunks, chunk_f, tag):
        t = pool.tile([P, n_chunks * chunk_f], f32, tag=tag)
        nc.sync.dma_start(
            out=t[:].rearrange("p (c f) -> p c f", c=n_chunks),
            in_=dram[:].rearrange("(c p) f -> p c f", p=P))
        return t

    covr = ld(din["cov"][:], [1, S], "covr")
    wcovr = ld(din["wcov"][:], [1, H], "wcovr")
    if variant == "fourier":
        encT_all = pb4.tile([P, KT * S], f32, tag="u2", name="encT_all")
    else:
        encT_all = big.tile([P, KT * S], f32, tag="bb", name="encT_all")
    nc.sync.dma_start(
        out=encT_all[:].rearrange("p (c f) -> p c f", c=KT),
        in_=din["encT"][:].rearrange("(c p) f -> p c f", p=P))
    encT = [encT_all[:, i * S:(i + 1) * S] for i in range(KT)]
    wc_all = ld_merged(pers, din["wc"], KT, H, "wc")
    wcr_all = pers.tile([P, KT * H], f32r, tag="wcr")
    nc.vector.tensor_copy(wcr_all[:], wc_all[:])
    wcr = [wcr_all[:, k * H:(k + 1) * H] for k in range(KT)]
    decT_all = ld_merged(pers, din["decT"], KT, T, "decT")
    decT = [decT_all[:, k * T:(k + 1) * T] for k in range(KT)]
    wq_all = ld_merged(pers, din["wq"], KT, H, "wq")
    wq = [wq_all[:, k * H:(k + 1) * H] for k in range(KT)]
    vwin_all = ld_merged(pers, din["vwin"].reshape([KT * P, 127]), KT, 127, "vwin")
    vwin = [vwin_all[:, k * 127:(k + 1) * 127] for k in range(KT)]
    bqr = ld(din["bq"][:], [1, H], "bqr")
    wo_all = ld_merged(pers, din["wo"], 2 * KT, H, "wo")
    wo = [wo_all[:, k * H:(k + 1) * H] for k in range(2 * KT)]
    bor = ld(din["bo"][:], [1, H], "bor")
    eye64 = ld(din["eye64"][:], [T, T], "eye64")
    ones64 = pers.tile([1, T], f32, tag="ones64")
    nc.vector.memset(ones64[:], 1.0)

    if PROBES:
        pr = ld(din["probe"][:], [P, T], "probe")
        ps = pers.tile([P, T], f32, tag="probe_sin")
        nc.scalar.activation(ps[:], pr[:], AF.Sin)
        nc.sync.dma_start(out=dout["probe_sin"][:], in_=ps[:])
        pm = pers.tile([P, T], f32, tag="probe_mod")
        ki = pers.tile([P, T], mybir.dt.int32, tag="probe_ki")
        nc.vector.tensor_scalar(ki[:], pr[:], float(1.0 / (2 * np.pi)), None, ALU.mult)
        nc.vector.tensor_copy(pm[:], ki[:])
        nc.sync.dma_start(out=dout["probe_mod"][:], in_=pm[:])

    # coverage: enc = tanh(enc_raw + cov (x) wcov) in both layouts.
    # The [S,H]-layout copy is only needed by the epilogue c-matmul, so its
    # DMA + coverage are emitted after the main loop (emit_enc below).
    def emit_enc():
        enc_all = (big.tile([P, KT * H], f32, tag="tb", name="enc_all")
                   if variant == "exact"
                   else pers.tile([P, KT * H], f32, tag="enc", name="enc_all"))
        nc.sync.dma_start(
            out=enc_all[:].rearrange("p (c f) -> p c f", c=KT),
            in_=din["enc"][:].rearrange("(c p) f -> p c f", p=P))
        enc = [enc_all[:, j * H:(j + 1) * H] for j in range(KT)]
        for j in range(KT):  # [S,H] layout: outer[s,h] = cov[s]*wcov[h]
            op = psT.tile([P, H], f32, tag="pt")
            nc.tensor.matmul(op[:], covr[0:1, j * P:(j + 1) * P], wcovr[0:1, :],
                             start=True, stop=True)
            nc.vector.tensor_add(enc[j], enc[j], op[:])
            nc.scalar.activation(enc[j], enc[j], AF.Tanh)
        return enc
    encT_t = big.tile([P, KT * S], f32r, tag="tb", name="encT_t")
    for i in range(KT):  # [H,S] layout
        op = psT.tile([P, S], f32, tag="pt")
        nc.tensor.matmul(op[:], wcovr[0:1, i * P:(i + 1) * P], covr[0:1, :],
                         start=True, stop=True)
        nc.vector.tensor_add(encT[i], encT[i], op[:])
        nc.scalar.activation(encT_t[:, i * S:(i + 1) * S], encT[i], AF.Tanh)

    # a2T[hout, s] = sum_hin Wc[hin,hout] * encT[hin,s]   (one [128, KT*S] tile)
    a2T = pers.tile([P, KT * S], f32, tag="a2T")
    for m in range(KT):
        pm2 = psT.tile([P, S], f32, tag="pt")
        for k in range(KT):
            nc.tensor.matmul(pm2[:], wcr[k][:, m * P:(m + 1) * P],
                             encT_t[:, k * S:(k + 1) * S],
                             start=(k == 0), stop=(k == KT - 1))
        nc.scalar.copy(a2T[:, m * S:(m + 1) * S], pm2[:])

    # a1T[hout, t] = sum_hin Wq[hin,hout] * decT[hin,t] + bq[hout]
    a1T = pers.tile([P, KT * T], f32, tag="a1T")
    for m in range(KT):
        pm1 = psSm.tile([P, T], f32, tag="ps")
        for k in range(KT):
            nc.tensor.matmul(pm1[:], wq[k][:, m * P:(m + 1) * P], decT[k][:],
                             start=(k == 0), stop=False)
        nc.tensor.matmul(pm1[:], bqr[0:1, m * P:(m + 1) * P], ones64[0:1, :],
                         start=False, stop=True)
        nc.scalar.copy(a1T[:, m * T:(m + 1) * T], pm1[:])

    scores = psOut.tile([T, S], f32, tag="out512")

    if variant == "exact":
        vwinr = []
        for k in range(KT):
            vr = pers.tile([P, 127], f32r, tag=f"vwinr{k}")
            nc.vector.tensor_copy(vr[:].bitcast(f32r), vwin[k][:])
            vwinr.append(vr)
        NG = T // TG
        first = True
        for g in range(NG):
            bb = big.tile([P, KT * TG * S], f32, tag="bb")
            for k in range(KT):
                for tt in range(TG):
                    t_ = g * TG + tt
                    seg = bb[:, (k * TG + tt) * S:(k * TG + tt + 1) * S]
                    nc.vector.tensor_scalar_add(
                        seg, a2T[:, k * S:(k + 1) * S],
                        a1T[:, k * T + t_:k * T + t_ + 1])
            tb = big.tile([P, KT * TG * S], f32r, tag="tb")
            nc.scalar.activation(tb[:], bb[:], AF.Tanh)
            for k in range(KT):
                for tt in range(TG):
                    t_ = g * TG + tt
                    seg = tb[:, (k * TG + tt) * S:(k * TG + tt + 1) * S]
                    last = (g == NG - 1 and k == KT - 1 and tt == TG - 1)
                    nc.tensor.matmul(scores[:], r(vwinr[k][:, 63 - t_:127 - t_]), seg,
                                     start=first, stop=last)
                    first = False
    else:  # fourier
        MF = len(FREQ_W)
        OFF = float(12 * np.pi)  # multiple of 2*pi; makes mod input positive
        HALF_PI = float(np.pi / 2)
        negpi = pers.tile([P, 1], f32, tag="negpi")
        nc.vector.memset(negpi[:], float(-np.pi))
        # a1-side features, batched over all frequencies: [128, MF*KT*T]
        FW = KT * T
        y1a = pb2.tile([P, MF * FW], f32, tag="tmp1")
        for i in range(MF):
            nc.vector.tensor_scalar(y1a[:, i * FW:(i + 1) * FW], a1T[:],
                                    float(FREQ_W[i]), OFF, ALU.mult, ALU.add)
        u1s = pb2.tile([P, MF * FW], f32, tag="tmp1")
        s1a = pers.tile([P, MF * FW], f32, tag="s1a")
        nc.vector.tensor_scalar(u1s[:], y1a[:], TWO_PI, None, ALU.mod)
        nc.scalar.activation(s1a[:], u1s[:], AF.Sin, bias=negpi[:])
        u1c = pb2.tile([P, MF * FW], f32, tag="tmp1")
        c1a = pers.tile([P, MF * FW], f32, tag="c1a")
        nc.vector.tensor_scalar(u1c[:], y1a[:], HALF_PI, TWO_PI, ALU.add, ALU.mod)
        nc.scalar.activation(c1a[:], u1c[:], AF.Sin, bias=negpi[:])
        for i in range(MF):
            wi = float(FREQ_W[i])
            bi = float(FREQ_B[i])
            # a2-side features (sign-flipped: sin(u - pi) = -sin(arg))
            y2 = pb2.tile([P, KT * S], f32, tag="y2")
            nc.vector.tensor_scalar(y2[:], a2T[:], wi, OFF, ALU.mult, ALU.add)
            u2s = pb4.tile([P, KT * S], f32, tag="u2")
            nc.vector.tensor_scalar(u2s[:], y2[:], TWO_PI, None, ALU.mod)
            s2 = pb4.tile([P, KT * S], f32, tag="f2")
            nc.scalar.activation(s2[:].bitcast(f32r), u2s[:], AF.Sin, bias=negpi[:])
            u2c = pb4.tile([P, KT * S], f32, tag="u2")
            nc.vector.tensor_scalar(u2c[:], y2[:], HALF_PI, TWO_PI, ALU.add, ALU.mod)
            c2 = pb4.tile([P, KT * S], f32, tag="f2")
            nc.scalar.activation(c2[:].bitcast(f32r), u2c[:], AF.Sin, bias=negpi[:])
            # scale a1 features by v[h]*b_i per hout chunk
            ws1 = pb4.tile([P, FW], f32, tag="wf1")
            wc1 = pb4.tile([P, FW], f32, tag="wf1")
            for k in range(KT):
                sl = slice(k * T, (k + 1) * T)
                nc.vector.tensor_scalar(ws1[:, sl].bitcast(f32r),
                                        s1a[:, i * FW + k * T:i * FW + (k + 1) * T],
                                        vwin[k][:, 63:64], bi, ALU.mult, ALU.mult)
                nc.vector.tensor_scalar(wc1[:, sl].bitcast(f32r),
                                        c1a[:, i * FW + k * T:i * FW + (k + 1) * T],
                                        vwin[k][:, 63:64], bi, ALU.mult, ALU.mult)
            for k in range(KT):
                nc.tensor.matmul(scores[:], r(ws1[:, k * T:(k + 1) * T]),
                                 r(c2[:, k * S:(k + 1) * S]),
                                 start=(i == 0 and k == 0), stop=False)
                nc.tensor.matmul(scores[:], r(wc1[:, k * T:(k + 1) * T]),
                                 r(s2[:, k * S:(k + 1) * S]),
                                 start=False, stop=(i == MF - 1 and k == KT - 1))

    enc = emit_enc()

    # softmax over s (free dim)
    negmax = pers.tile([T, 1], f32, tag="negmax")
    nc.vector.tensor_reduce(negmax[:], scores[:], axis=AX.X, op=ALU.max, negate=True)
    align_sb = pers.tile([T, S], f32, tag="align_sb")
    sums = pers.tile([T, 1], f32, tag="sums")
    nc.scalar.activation(align_sb[:], scores[:], AF.Exp, bias=negmax[:],
                         accum_out=sums[:])
    recips = pers.tile([T, 1], f32, tag="recips")
    nc.vector.reciprocal(recips[:], sums[:])
    nc.vector.tensor_scalar_mul(align_sb[:], align_sb[:], recips[:])
    nc.sync.dma_start(out=dout["align"][:], in_=align_sb[:])

    # alignT via PE transpose
    alignT = []
    for j in range(KT):
        pt = psSm.tile([P, T], f32, tag="ps")
        nc.tensor.transpose(pt[:], align_sb[:, j * P:(j + 1) * P], eye64[:])
        at = pers.tile([P, T], f32, tag=f"alignT{j}")
        nc.vector.tensor_copy(at[:], pt[:])
        alignT.append(at)

    # cT[h, t] = sum_s enc[s,h] * alignT[s,t]
    cT = []
    for m in range(KT):
        pc = psSm.tile([P, T], f32, tag="ps")
        for j in range(KT):
            nc.tensor.matmul(pc[:], enc[j][:, m * P:(m + 1) * P], alignT[j][:],
                             start=(j == 0), stop=(j == KT - 1))
        ct = pers.tile([P, T], f32, tag=f"cT{m}")
        nc.vector.tensor_copy(ct[:], pc[:])
        cT.append(ct)

    # attn_h = [c, dec] @ Wo + bo
    pa = psOut.tile([T, H], f32, tag="out512")
    for k in range(KT):
        nc.tensor.matmul(pa[:], cT[k][:], wo[k][:], start=(k == 0), stop=False)
    for k in range(KT):
        nc.tensor.matmul(pa[:], decT[k][:], wo[KT + k][:], start=False, stop=False)
    nc.tensor.matmul(pa[:], ones64[0:1, :], bor[0:1, :], start=False, stop=True)
    attn_sb = pers.tile([T, H], f32, tag="attn_sb")
    nc.vector.tensor_copy(attn_sb[:], pa[:])
    nc.sync.dma_start(out=dout["attn_h"][:], in_=attn_sb[:])


def build(variant=None):
    variant = variant or VARIANT
    if variant in _BUILT:
        return _BUILT[variant]
    from contextlib import ExitStack

    import concourse.bacc as bacc
    import concourse.mybir as mybir
    import concourse.tile as tile

    f32 = mybir.dt.float32
    nc = bacc.Bacc("TRN2", target_bir_lowering=False, debug=False)
    in_specs = [
        ("decT", [H, T]), ("enc", [S, H]), ("encT", [H, S]),
        ("wq", [H, H]), ("wc", [H, H]), ("wo", [2 * H, H]),
        ("vwin", [KT, P, 127]), ("cov", [1, S]), ("wcov", [1, H]),
        ("bq", [1, H]), ("bo", [1, H]), ("eye64", [T, T]),
    ]
    out_specs = [("attn_h", [T, H]), ("align", [T, S])]
    if PROBES:
        in_specs.append(("probe", [P, T]))
        out_specs += [("probe_sin", [P, T]), ("probe_mod", [P, T])]
    din = {n: nc.declare_dram_parameter(n, s, f32, isOutput=False) for n, s in in_specs}
    dout = {n: nc.declare_dram_parameter(n, s, f32, isOutput=True) for n, s in out_specs}
    with ExitStack() as ctx:
        tc = ctx.enter_context(tile.TileContext(nc))
        _emit(nc, tc, ctx, din, dout, variant)
    nc.compile()
    _BUILT[variant] = nc
    return nc


def prep_core_inputs(inputs):
    """Host-side shard: per-core input dicts (core b <- batch element b)."""
    dec = np.asarray(inputs["attn_dec_state"], np.float32)  # [T,B,H]
    encr = np.asarray(inputs["attn_enc_state"], np.float32)  # [S,B,H]
    cov = np.asarray(inputs["attn_coverage"], np.float32)  # [B,S]
    Wq = np.ascontiguousarray(np.asarray(inputs["Wq"], np.float32))
    Wc = np.ascontiguousarray(np.asarray(inputs["Wc"], np.float32))
    Wo = np.ascontiguousarray(np.asarray(inputs["Wo"], np.float32))
    v = np.asarray(inputs["v"], np.float32)
    bq = np.asarray(inputs["bq"], np.float32)[None, :]
    bo = np.asarray(inputs["bo"], np.float32)[None, :]
    wcov = np.asarray(inputs["wcov"], np.float32)[None, :]
    vwin = np.zeros((KT, P, 127), np.float32)
    for k in range(KT):
        vwin[k, :, 63] = v[k * P:(k + 1) * P]
    eye64 = np.eye(T, dtype=np.float32)
    shared = dict(wq=Wq, wc=Wc, wo=Wo, vwin=vwin, wcov=wcov, bq=bq, bo=bo,
                  eye64=eye64)
    if PROBES:
        shared["probe"] = np.linspace(-16, 16, P * T).astype(np.float32).reshape(P, T)
    maps = []
    for b in range(B):
        e = np.ascontiguousarray(encr[:, b, :])
        maps.append(dict(
            decT=np.ascontiguousarray(dec[:, b, :].T),
            enc=e,
            encT=np.ascontiguousarray(e.T),
            cov=np.ascontiguousarray(cov[b][None, :]),
            **shared,
        ))
    return maps


def kernel(**inputs):
    global LAST_RESULT
    nc = build()
    in_maps = prep_core_inputs(inputs)
    from concourse.bass_utils import run_bass_kernel_spmd

    trace = os.environ.get("ATTN_TRACE", "0") == "1"
    res = run_bass_kernel_spmd(nc, in_maps, list(range(B)), trace=trace)
    LAST_RESULT = res
    attn_h = np.stack([res.results[i]["attn_h"] for i in range(B)], axis=1)
    align = np.stack([res.results[i]["align"] for i in range(B)], axis=1)
    return attn_h, align
